# revision 16
# baseline (speedup 1.0000x reference)
"""Trainium2 Bass kernel for nn_ExLRestSelfAtten (windowed self-attention).

Reference computation (per batch b of 256, seq S=100, window 11):
    h   = relu(x @ W1 + b1)                       [B,100,128]
    hp  = zero-pad h to [B,110,128]  (5 each side)
    x_nei[b,s,j] = hp[b, s+10-j]                  (11 shifted copies)
    q/k/v = x_nei @ Wq/Wk/Wv
    scores[b,s,i,j] = q[b,s,i] . k[b,s,j] / sqrt(128)
    alpha = softmax_j(scores)                     [B,100,11,11]
    pooled[b,s] = sum_i sum_j alpha[s,i,j] v[b,s,j]
    out = sigmoid(pooled @ W2 + b2)               [B,100,2]
    returns (out, alpha.reshape(B*100,11,11))

Key identity used here: q[b,s,i] = qh[b, s+10-i] where qh = hp @ Wq (same for
k,v) - so everything reduces to per-batch 110x110 banded Gram matrices plus
Toeplitz-style gathers, instead of materializing [B,100,11,128] tensors.

Per batch (all on-chip except two small DRAM round-trips for the diagonal
re-layouts, which compute engines cannot express):
    xT      = PE-transpose of x chunks                      (3x [100,100])
    hT      = W1c.T-accumulated matmuls -> relu(+b1) -> hpT [128,110] (padded)
    qhT/khT = Wq/Wk.T @ hpT                                 [128,110]
    vS      = hpT.T @ Wv  (lhsT=hpT trick -> transposed V)  [110,128]
    G       = qhT.T @ khT                                   [110,110] PSUM
    E       = exp(G/sqrt(128)) on PSUM-evict                [110,110]
    E -> DRAM; gather S[s,i,j] = E[s+10-i, s+10-j]          [100,11,11]
    D[s,i] = sum_j S; alpha = S * (1/D)  -> DRAM output
    beta[s,j] = sum_i alpha; scatter to banded GammaT[r,s]=beta[s,10-r+s]
    pooledT = vS.T @ GammaT                                 [128,100]
    outT    = W2.T @ pooledT                                [2,100]
    (sigmoid + b2 applied once at end of core over all batches)
"""

import numpy as np

import concourse.bass as bass
import concourse.bacc as bacc
import concourse.mybir as mybir
import concourse.tile as tile
from concourse.bass_types import dram_disjoint_views
from concourse.masks import make_identity

F32 = mybir.dt.float32

B = 256          # total batches
NCORES = 8
BPC = B // NCORES  # 32 batches per core
S = 100          # sequence length
SP = 110         # padded sequence length
A = 5            # atten half-window
W = 11           # window size
HID = 128
CIN = 300
OUT = 2
NBUF = 4         # DRAM scratch rotation depth
INV_SQRT_H = 1.0 / float(np.sqrt(128.0))


def _build_kernel():
    nc = bacc.Bacc(
        "TRN2", target_bir_lowering=False, debug=False, num_devices=NCORES
    )

    x = nc.dram_tensor("x", [BPC, S, CIN], F32, kind="ExternalInput")
    w1 = nc.dram_tensor("w1", [CIN, HID], F32, kind="ExternalInput")
    b1 = nc.dram_tensor("b1", [HID], F32, kind="ExternalInput")
    wq = nc.dram_tensor("wq", [HID, HID], F32, kind="ExternalInput")
    wk = nc.dram_tensor("wk", [HID, HID], F32, kind="ExternalInput")
    wv = nc.dram_tensor("wv", [HID, HID], F32, kind="ExternalInput")
    w2 = nc.dram_tensor("w2", [HID, OUT], F32, kind="ExternalInput")
    b2 = nc.dram_tensor("b2", [OUT], F32, kind="ExternalInput")

    out = nc.dram_tensor("out", [BPC, S, OUT], F32, kind="ExternalOutput")
    alpha = nc.dram_tensor("alpha", [BPC * S, W, W], F32, kind="ExternalOutput")

    # DRAM scratch for the two diagonal re-layouts (rotated NBUF-deep).
    ed = nc.dram_tensor("ed_scratch", [NBUF, SP, SP], F32)
    gtd = nc.dram_tensor("gtd_scratch", [NBUF, SP, S], F32)
    ed_v = [v[n] for n, v in enumerate(dram_disjoint_views(ed[:], NBUF))]
    gtd_v = [v[n] for n, v in enumerate(dram_disjoint_views(gtd[:], NBUF))]

    with tile.TileContext(nc) as tc:
        with (
            tc.tile_pool(name="const", bufs=1) as cpool,
            tc.tile_pool(name="work", bufs=3) as wp,
            tc.tile_pool(name="acc", bufs=1) as accp,
            tc.tile_pool(name="ps", bufs=8, space="PSUM") as pp,
        ):
            # ---- constants (loaded once) ----
            ident = cpool.tile([128, 128], F32)
            make_identity(nc, ident)
            w1_sb = cpool.tile([100, 3, HID], F32)
            for k in range(3):
                nc.sync.dma_start(out=w1_sb[:, k, :], in_=w1[k * 100:(k + 1) * 100, :])
            wq_sb = cpool.tile([HID, HID], F32)
            wk_sb = cpool.tile([HID, HID], F32)
            wv_sb = cpool.tile([HID, HID], F32)
            nc.sync.dma_start(out=wq_sb, in_=wq[:])
            nc.sync.dma_start(out=wk_sb, in_=wk[:])
            nc.sync.dma_start(out=wv_sb, in_=wv[:])
            w2_sb = cpool.tile([HID, OUT], F32)
            nc.sync.dma_start(out=w2_sb, in_=w2[:])
            b1_sb = cpool.tile([HID, 1], F32)
            nc.sync.dma_start(out=b1_sb, in_=b1[:].unsqueeze(-1))
            b2_sb = cpool.tile([S, OUT], F32)
            nc.gpsimd.dma_start(
                out=b2_sb,
                in_=bass.AP(tensor=b2[:].tensor, offset=0, ap=[[0, S], [1, OUT]]),
            )

            # zero the banded-scatter scratch once (band complement must stay 0)
            zt = cpool.tile([SP, S], F32)
            nc.vector.memset(zt, 0.0)
            for n in range(NBUF):
                nc.gpsimd.dma_start(out=gtd_v[n], in_=zt)

            # per-core output accumulators (sigmoid applied once at the end)
            outacc = accp.tile([S, BPC, OUT], F32)
            outacc2 = accp.tile([S, BPC, OUT], F32)

            for b in range(BPC):
                slot = b % NBUF

                # ---- stage A: load x, transpose, first matmul ----
                x_sb = wp.tile([S, CIN], F32, tag="x")
                nc.sync.dma_start(out=x_sb, in_=x[b])

                xt_sbs = []
                for k in range(3):
                    xtp = pp.tile([100, 100], F32, tag="ps")
                    nc.tensor.matmul(
                        xtp, lhsT=x_sb[:, k * 100:(k + 1) * 100],
                        rhs=ident[:100, :100], start=True, stop=True,
                    )
                    xt_sb = wp.tile([100, 100], F32, tag=f"xt{k}")
                    if k == 0:
                        nc.vector.tensor_copy(out=xt_sb, in_=xtp)
                    else:
                        nc.scalar.copy(out=xt_sb, in_=xtp)
                    xt_sbs.append(xt_sb)

                htp = pp.tile([HID, S], F32, tag="ps")
                for k in range(3):
                    nc.tensor.matmul(
                        htp, lhsT=w1_sb[:, k, :], rhs=xt_sbs[k],
                        start=(k == 0), stop=(k == 2),
                    )

                hpt = wp.tile([HID, SP], F32, tag="hpt")
                nc.gpsimd.memset(hpt[:, 0:A], 0.0)
                nc.gpsimd.memset(hpt[:, S + A:SP], 0.0)
                nc.scalar.activation(
                    out=hpt[:, A:S + A], in_=htp,
                    func=mybir.ActivationFunctionType.Relu,
                    bias=b1_sb, scale=1.0,
                )

                # ---- stage B: qkv + Gram + exp ----
                qhp = pp.tile([HID, SP], F32, tag="ps")
                nc.tensor.matmul(qhp, lhsT=wq_sb, rhs=hpt, start=True, stop=True)
                qh_sb = wp.tile([HID, SP], F32, tag="qh")
                nc.scalar.copy(out=qh_sb, in_=qhp)

                khp = pp.tile([HID, SP], F32, tag="ps")
                nc.tensor.matmul(khp, lhsT=wk_sb, rhs=hpt, start=True, stop=True)
                kh_sb = wp.tile([HID, SP], F32, tag="kh")
                nc.vector.tensor_copy(out=kh_sb, in_=khp)

                vsp = pp.tile([SP, HID], F32, tag="ps")
                nc.tensor.matmul(vsp, lhsT=hpt, rhs=wv_sb, start=True, stop=True)
                vs_sb = wp.tile([SP, HID], F32, tag="vs")
                nc.vector.tensor_copy(out=vs_sb, in_=vsp)

                gp = pp.tile([SP, SP], F32, tag="ps")
                nc.tensor.matmul(gp, lhsT=qh_sb, rhs=kh_sb, start=True, stop=True)
                e_sb = wp.tile([SP, SP], F32, tag="e")
                nc.scalar.activation(
                    out=e_sb, in_=gp,
                    func=mybir.ActivationFunctionType.Exp, scale=INV_SQRT_H,
                )

                # ---- stage C: diagonal gather via DRAM, softmax, scatter ----
                nc.sync.dma_start(out=ed_v[slot], in_=e_sb)

                # S'[s,i,j'] = E[s+10-i, s+j']  (j' = 10-j, ascending so the
                # DMA's final dim is forward-contiguous; un-reversed at the
                # normalization multiply below via a negative-step read AP)
                s_sb = wp.tile([S, W, W], F32, tag="s")
                gather_src = bass.AP(
                    tensor=ed_v[slot].tensor,
                    offset=ed_v[slot].offset + (2 * A) * SP,
                    ap=[[SP + 1, S], [-SP, W], [1, W]],
                )
                nc.sync.dma_start(out=s_sb, in_=gather_src, max_dma_last_dim=W)

                d_sb = wp.tile([S, W], F32, tag="d")
                nc.vector.reduce_sum(out=d_sb, in_=s_sb, axis=mybir.AxisListType.X)
                dr_sb = wp.tile([S, W], F32, tag="dr")
                nc.vector.reciprocal(out=dr_sb, in_=d_sb)

                al_sb = wp.tile([S, W, W], F32, tag="al")
                s_rev = bass.AP(
                    tensor=s_sb[:].tensor,
                    offset=s_sb[:].offset + (W - 1),
                    ap=[s_sb[:].ap[0], [W, W], [-1, W]],
                )
                nc.vector.tensor_mul(
                    out=al_sb, in0=s_rev,
                    in1=dr_sb[:].unsqueeze(-1).broadcast_to([S, W, W]),
                )
                nc.sync.dma_start(out=alpha[b * S:(b + 1) * S], in_=al_sb)

                beta_sb = wp.tile([S, W], F32, tag="beta")
                nc.vector.reduce_sum(
                    out=beta_sb, in_=al_sb[:].rearrange("p i j -> p j i"),
                    axis=mybir.AxisListType.X,
                )

                # scatter beta[s,j] -> GammaT[s+10-j, s] (band of [110,100])
                scatter_dst = bass.AP(
                    tensor=gtd_v[slot].tensor,
                    offset=gtd_v[slot].offset + 2 * A * S,
                    ap=[[S + 1, S], [-S, W], [1, 1]],
                )
                nc.sync.dma_start(out=scatter_dst, in_=beta_sb)

                gt_sb = wp.tile([SP, S], F32, tag="gt")
                nc.sync.dma_start(out=gt_sb, in_=gtd_v[slot])

                # ---- stage D: weighted pool + head ----
                ptp = pp.tile([HID, S], F32, tag="ps")
                nc.tensor.matmul(ptp, lhsT=vs_sb, rhs=gt_sb, start=True, stop=True)
                pt_sb = wp.tile([HID, S], F32, tag="pt")
                nc.scalar.copy(out=pt_sb, in_=ptp)

                outp = pp.tile([S, OUT], F32, tag="ps")
                nc.tensor.matmul(outp, lhsT=pt_sb, rhs=w2_sb, start=True, stop=True)
                nc.vector.tensor_copy(out=outacc[:, b, :], in_=outp)

            # ---- epilogue: +b2, sigmoid once, single out DMA ----
            nc.vector.tensor_add(
                out=outacc, in0=outacc,
                in1=bass.AP(
                    tensor=b2_sb[:].tensor, offset=b2_sb[:].offset,
                    ap=[b2_sb[:].ap[0], [0, BPC], [1, OUT]],
                ),
            )
            nc.scalar.activation(
                out=outacc2, in_=outacc,
                func=mybir.ActivationFunctionType.Sigmoid,
            )
            out_dst = bass.AP(
                tensor=out[:].tensor, offset=0,
                ap=[[OUT, S], [S * OUT, BPC], [1, OUT]],
            )
            nc.sync.dma_start(out=out_dst, in_=outacc2)

    nc.compile()
    return nc


_NC_CACHE = None


def kernel(x, W1, b1, Wq, Wk, Wv, W2, b2):
    global _NC_CACHE
    if _NC_CACHE is None:
        _NC_CACHE = _build_kernel()
    nc = _NC_CACHE

    from concourse.bass_utils import run_bass_kernel_spmd

    x = np.ascontiguousarray(np.asarray(x, dtype=np.float32))
    in_maps = []
    for c in range(NCORES):
        in_maps.append({
            "x": x[c * BPC:(c + 1) * BPC],
            "w1": np.ascontiguousarray(np.asarray(W1, np.float32)),
            "b1": np.ascontiguousarray(np.asarray(b1, np.float32).reshape(HID)),
            "wq": np.ascontiguousarray(np.asarray(Wq, np.float32)),
            "wk": np.ascontiguousarray(np.asarray(Wk, np.float32)),
            "wv": np.ascontiguousarray(np.asarray(Wv, np.float32)),
            "w2": np.ascontiguousarray(np.asarray(W2, np.float32)),
            "b2": np.ascontiguousarray(np.asarray(b2, np.float32).reshape(OUT)),
        })

    res = run_bass_kernel_spmd(nc, in_maps, core_ids=list(range(NCORES)))
    out_full = np.concatenate([res.results[c]["out"] for c in range(NCORES)], axis=0)
    alpha_full = np.concatenate(
        [res.results[c]["alpha"] for c in range(NCORES)], axis=0
    )
    return out_full, alpha_full


# revision 22
# speedup vs baseline: 3.3275x; 3.3275x over previous
"""Trainium2 Bass kernel for nn_ExLRestSelfAtten (windowed self-attention).

Reference computation (per batch b of 256, seq S=100, window 11):
    h   = relu(x @ W1 + b1)                       [B,100,128]
    hp  = zero-pad h to [B,110,128]  (5 each side)
    x_nei[b,s,j] = hp[b, s+10-j]                  (11 shifted copies)
    q/k/v = x_nei @ Wq/Wk/Wv
    scores[b,s,i,j] = q[b,s,i] . k[b,s,j] / sqrt(128)
    alpha = softmax_j(scores)                     [B,100,11,11]
    pooled[b,s] = sum_i sum_j alpha[s,i,j] v[b,s,j]
    out = sigmoid(pooled @ W2 + b2)               [B,100,2]
    returns (out, alpha.reshape(B*100,11,11))

Key identity: q[b,s,i] = qh[b, s+10-i] where qh = hp @ Wq (same for k,v) -
everything reduces to per-batch 110x110 banded Gram matrices plus
Toeplitz-style gathers, instead of materializing [B,100,11,128] tensors.

Batches are processed in groups of GRP=4 so the shared-weight matmuls
(x@W1, Wq/Wk against hp) run with a 400+-wide moving dim - the float32r
1-cycle/row fast path. Per batch within a group (x arrives host-transposed
and group-packed as [G, 300, GRP*100]):
    hT      = W1c.T-accumulated matmuls -> +b1,relu -> hpT  [128,GRP,110]
    qhT/khT = Wq/Wk.T @ hpT                                 [128,GRP*110]
    vS      = hpT.T @ Wv  (lhsT=hpT trick -> transposed V)  [110,128]
    G       = qhT.T @ khT                                   [110,110] PSUM
    E       = exp(G/sqrt(128)) on PSUM-evict                [110,110]
    E -> DRAM; gather S'[s,i,j'] = E[s+10-i, s+j']          [100,11,11]
    D[s,i] = sum_j' S'; alpha = rev_j(S') * (1/D)           (group DMA out)
    beta[s,j] = sum_i alpha; scatter to banded GammaT[s+10-j, s] = beta[s,j]
    pooledT = vS.T @ GammaT                                 [128,100]
    out     = pooledT.T @ W2                                [100,2]
    (+b2, sigmoid applied once at end of core over all batches)
"""

import numpy as np

import concourse.bass as bass
import concourse.bacc as bacc
import concourse.mybir as mybir
import concourse.tile as tile
from concourse.bass_types import dram_disjoint_views

F32 = mybir.dt.float32
F32R = mybir.dt.float32r if __import__('os').environ.get('USE_F32R','1')=='1' else mybir.dt.float32

B = 256          # total batches
NCORES = 8
BPC = B // NCORES  # 32 batches per core
GRP = 4          # batches per matmul group
NG = BPC // GRP  # groups per core
S = 100          # sequence length
SP = 110         # padded sequence length
A = 5            # atten half-window
W = 11           # window size
HID = 128
CIN = 300
OUT = 2
NBUF = 8         # DRAM scratch rotation depth
INV_SQRT_H = 1.0 / float(np.sqrt(128.0))


def _mm(nc, out, lhsT, rhs, **kw):
    nc.tensor.matmul(out, lhsT=lhsT, rhs=rhs, **kw)


def _build_kernel():
    nc = bacc.Bacc(
        "TRN2", target_bir_lowering=False, debug=False, num_devices=NCORES
    )

    # host-packed: xt[g, c, b'*100 + s] = x[g*GRP + b', s, c]
    xt = nc.dram_tensor("xt", [NG, CIN, GRP * S], F32, kind="ExternalInput")
    w1 = nc.dram_tensor("w1", [CIN, HID], F32, kind="ExternalInput")
    b1 = nc.dram_tensor("b1", [HID], F32, kind="ExternalInput")
    wq = nc.dram_tensor("wq", [HID, HID], F32, kind="ExternalInput")
    wk = nc.dram_tensor("wk", [HID, HID], F32, kind="ExternalInput")
    wv = nc.dram_tensor("wv", [HID, HID], F32, kind="ExternalInput")
    w2 = nc.dram_tensor("w2", [HID, OUT], F32, kind="ExternalInput")
    b2 = nc.dram_tensor("b2", [OUT], F32, kind="ExternalInput")

    out = nc.dram_tensor("out", [BPC, S, OUT], F32, kind="ExternalOutput")
    alpha = nc.dram_tensor("alpha", [BPC * S, W, W], F32, kind="ExternalOutput")

    # DRAM scratch for the two diagonal re-layouts (rotated NBUF-deep).
    ed = nc.dram_tensor("ed_scratch", [NBUF, SP, SP], F32)
    gtd = nc.dram_tensor("gtd_scratch", [NBUF, SP, S], F32)
    ed_v = [v[n] for n, v in enumerate(dram_disjoint_views(ed[:], NBUF))]
    gtd_v = [v[n] for n, v in enumerate(dram_disjoint_views(gtd[:], NBUF))]

    with tile.TileContext(nc) as tc:
        with (
            tc.tile_pool(name="const", bufs=1) as cpool,
            tc.tile_pool(name="grp", bufs=3) as gp_pool,
            tc.tile_pool(name="work", bufs=6) as wp,
            tc.tile_pool(name="acc", bufs=1) as accp,
            tc.tile_pool(name="ps1", bufs=1, space="PSUM") as pp1,
            tc.tile_pool(name="ps2", bufs=2, space="PSUM") as pp2,
        ):
            # ---- constants (loaded once; matmul inputs cast to f32r) ----
            w1_sb = cpool.tile([100, 3, HID], F32R)
            for k in range(3):
                nc.gpsimd.dma_start(
                    out=w1_sb[:, k, :], in_=w1[k * 100:(k + 1) * 100, :]
                )
            wq_sb = cpool.tile([HID, HID], F32R)
            wk_sb = cpool.tile([HID, HID], F32R)
            wv_sb = cpool.tile([HID, HID], F32R)
            nc.gpsimd.dma_start(out=wq_sb, in_=wq[:])
            nc.gpsimd.dma_start(out=wk_sb, in_=wk[:])
            nc.gpsimd.dma_start(out=wv_sb, in_=wv[:])
            w2_sb = cpool.tile([HID, OUT], F32R)
            nc.gpsimd.dma_start(out=w2_sb, in_=w2[:])
            b1_sb = cpool.tile([HID, 1], F32)
            nc.sync.dma_start(out=b1_sb, in_=b1[:].unsqueeze(-1))
            b2_sb = cpool.tile([S, OUT], F32)
            nc.gpsimd.dma_start(
                out=b2_sb,
                in_=bass.AP(tensor=b2[:].tensor, offset=0, ap=[[0, S], [1, OUT]]),
            )

            # zero the banded-scatter scratch once (band complement must stay 0)
            zt = cpool.tile([SP, S], F32)
            nc.vector.memset(zt, 0.0)
            for n in range(NBUF):
                nc.gpsimd.dma_start(out=gtd_v[n], in_=zt)

            # per-core output accumulators (sigmoid applied once at the end)
            outacc = accp.tile([S, BPC, OUT], F32)
            outacc2 = accp.tile([S, BPC, OUT], F32)

            for g in range(NG):
                # ---- group stage: load xT, mm1, bias+relu, qk ----
                xt_sb = gp_pool.tile([100, 3, GRP * S], F32R, tag="x")
                # src iterated (c', k, s~) to match dest dim order
                xt_src = bass.AP(
                    tensor=xt[:].tensor, offset=g * CIN * GRP * S,
                    ap=[[GRP * S, 100], [100 * GRP * S, 3], [1, GRP * S]],
                )
                nc.gpsimd.dma_start(out=xt_sb, in_=xt_src)

                htp = pp1.tile([HID, GRP * S], F32, tag="htp")
                for k in range(3):
                    _mm(nc, htp, w1_sb[:, k, :], xt_sb[:, k, :],
                        start=(k == 0), stop=(k == 2))

                hpt = gp_pool.tile([HID, GRP, SP], F32R, tag="hpt")
                for bi in range(GRP):
                    nc.gpsimd.memset(hpt[:, bi, 0:A].bitcast(F32), 0.0)
                    nc.gpsimd.memset(hpt[:, bi, S + A:SP].bitcast(F32), 0.0)
                # relu(h + b1): one DVE op, (x add b1) max 0
                nc.vector.tensor_scalar(
                    out=hpt[:, :, A:S + A],
                    in0=htp[:].rearrange("p (g s) -> p g s", g=GRP),
                    scalar1=b1_sb, scalar2=0.0,
                    op0=mybir.AluOpType.add, op1=mybir.AluOpType.max,
                )
                hpt_flat = hpt[:].rearrange("p g s -> p (g s)")

                qhp = pp1.tile([HID, GRP * SP], F32, tag="qhp")
                _mm(nc, qhp, wq_sb[:], hpt_flat, start=True, stop=True)
                qh_sb = gp_pool.tile([HID, GRP * SP], F32R, tag="qh")
                nc.scalar.copy(out=qh_sb, in_=qhp)

                khp = pp1.tile([HID, GRP * SP], F32, tag="khp")
                _mm(nc, khp, wk_sb[:], hpt_flat, start=True, stop=True)
                kh_sb = gp_pool.tile([HID, GRP * SP], F32R, tag="kh")
                nc.vector.tensor_copy(out=kh_sb, in_=khp)

                al_g = gp_pool.tile([S, GRP, W * W], F32, tag="al")

                for bi in range(GRP):
                    b = g * GRP + bi
                    slot = b % NBUF

                    # ---- per-batch: vS, Gram, exp ----
                    vsp = pp1.tile([SP, HID], F32, tag="vsp")
                    _mm(nc, vsp, hpt[:, bi, :], wv_sb[:], start=True, stop=True)
                    vs_sb = wp.tile([SP, HID], F32R, tag="vs")
                    nc.vector.tensor_copy(out=vs_sb, in_=vsp)

                    gpp = pp2.tile([SP, SP], F32, tag="gp")
                    _mm(nc, gpp,
                        qh_sb[:, bi * SP:(bi + 1) * SP],
                        kh_sb[:, bi * SP:(bi + 1) * SP],
                        start=True, stop=True)
                    e_sb = wp.tile([SP, SP], F32, tag="e")
                    nc.scalar.activation(
                        out=e_sb, in_=gpp,
                        func=mybir.ActivationFunctionType.Exp, scale=INV_SQRT_H,
                    )

                    # ---- diagonal gather via DRAM, softmax, scatter ----
                    nc.scalar.dma_start(out=ed_v[slot], in_=e_sb)

                    # S'[s,i,j'] = E[s+10-i, s+j']  (j'=10-j, fwd-contiguous)
                    s_sb = wp.tile([S, W, W], F32, tag="s")
                    gather_src = bass.AP(
                        tensor=ed_v[slot].tensor,
                        offset=ed_v[slot].offset + (2 * A) * SP,
                        ap=[[SP + 1, S], [-SP, W], [1, W]],
                    )
                    nc.scalar.dma_start(out=s_sb, in_=gather_src,
                                        max_dma_last_dim=W)

                    d_sb = wp.tile([S, W], F32, tag="d")
                    nc.vector.reduce_sum(out=d_sb, in_=s_sb,
                                         axis=mybir.AxisListType.X)
                    dr_sb = wp.tile([S, W], F32, tag="dr")
                    nc.vector.reciprocal(out=dr_sb, in_=d_sb)

                    s_rev = bass.AP(
                        tensor=s_sb[:].tensor,
                        offset=s_sb[:].offset + (W - 1),
                        ap=[s_sb[:].ap[0], [W, W], [-1, W]],
                    )
                    nc.vector.tensor_mul(
                        out=al_g[:, bi, :].rearrange("p (i j) -> p i j", i=W),
                        in0=s_rev,
                        in1=dr_sb[:].unsqueeze(-1).broadcast_to([S, W, W]),
                    )

                    beta_sb = wp.tile([S, W], F32, tag="beta")
                    nc.vector.reduce_sum(
                        out=beta_sb,
                        in_=al_g[:, bi, :].rearrange("p (i j) -> p j i", i=W),
                        axis=mybir.AxisListType.X,
                    )

                    # scatter beta[s,j] -> GammaT[s+10-j, s] (band of [110,100])
                    scatter_dst = bass.AP(
                        tensor=gtd_v[slot].tensor,
                        offset=gtd_v[slot].offset + 2 * A * S,
                        ap=[[S + 1, S], [-S, W], [1, 1]],
                    )
                    nc.gpsimd.dma_start(out=scatter_dst, in_=beta_sb)

                    gt_sb = wp.tile([SP, S], F32R, tag="gt")
                    nc.gpsimd.dma_start(out=gt_sb, in_=gtd_v[slot])

                    # ---- weighted pool + head ----
                    ptp = pp1.tile([HID, S], F32, tag="ptp")
                    _mm(nc, ptp, vs_sb[:], gt_sb[:], start=True, stop=True)
                    pt_sb = wp.tile([HID, S], F32R, tag="pt")
                    nc.scalar.copy(out=pt_sb, in_=ptp)

                    outp = pp1.tile([S, OUT], F32, tag="outp")
                    _mm(nc, outp, pt_sb[:], w2_sb[:], start=True, stop=True)
                    nc.vector.tensor_copy(out=outacc[:, b, :], in_=outp)

                # group alpha DMA: [S, GRP, 121] -> alpha[g*GRP*S ...]
                # dest element (s, bi, ij) at flat ((g*GRP+bi)*S + s)*121 + ij
                al_dst = bass.AP(
                    tensor=alpha[:].tensor,
                    offset=g * GRP * S * W * W,
                    ap=[[W * W, S], [S * W * W, GRP], [1, W * W]],
                )
                nc.sync.dma_start(out=al_dst, in_=al_g)

            # ---- epilogue: +b2, sigmoid once, single out DMA ----
            nc.vector.tensor_add(
                out=outacc, in0=outacc,
                in1=bass.AP(
                    tensor=b2_sb[:].tensor, offset=b2_sb[:].offset,
                    ap=[b2_sb[:].ap[0], [0, BPC], [1, OUT]],
                ),
            )
            nc.scalar.activation(
                out=outacc2, in_=outacc,
                func=mybir.ActivationFunctionType.Sigmoid,
            )
            out_dst = bass.AP(
                tensor=out[:].tensor, offset=0,
                ap=[[OUT, S], [S * OUT, BPC], [1, OUT]],
            )
            nc.sync.dma_start(out=out_dst, in_=outacc2)

    nc.compile()
    return nc


_NC_CACHE = None


def _pack_x(x_shard):
    # [BPC, S, CIN] -> [NG, CIN, GRP*S]: xt[g, c, b'*S + s] = x[g*GRP+b', s, c]
    return np.ascontiguousarray(
        x_shard.reshape(NG, GRP, S, CIN).transpose(0, 3, 1, 2).reshape(
            NG, CIN, GRP * S
        )
    )


def kernel(x, W1, b1, Wq, Wk, Wv, W2, b2):
    global _NC_CACHE
    if _NC_CACHE is None:
        _NC_CACHE = _build_kernel()
    nc = _NC_CACHE

    from concourse.bass_utils import run_bass_kernel_spmd

    x = np.asarray(x, dtype=np.float32)
    in_maps = []
    for c in range(NCORES):
        in_maps.append({
            "xt": _pack_x(x[c * BPC:(c + 1) * BPC]),
            "w1": np.ascontiguousarray(np.asarray(W1, np.float32)),
            "b1": np.ascontiguousarray(np.asarray(b1, np.float32).reshape(HID)),
            "wq": np.ascontiguousarray(np.asarray(Wq, np.float32)),
            "wk": np.ascontiguousarray(np.asarray(Wk, np.float32)),
            "wv": np.ascontiguousarray(np.asarray(Wv, np.float32)),
            "w2": np.ascontiguousarray(np.asarray(W2, np.float32)),
            "b2": np.ascontiguousarray(np.asarray(b2, np.float32).reshape(OUT)),
        })

    res = run_bass_kernel_spmd(nc, in_maps, core_ids=list(range(NCORES)))
    out_full = np.concatenate([res.results[c]["out"] for c in range(NCORES)], axis=0)
    alpha_full = np.concatenate(
        [res.results[c]["alpha"] for c in range(NCORES)], axis=0
    )
    return out_full, alpha_full
